# revision 7
# baseline (speedup 1.0000x reference)
"""CondLaneNet head kernel for Trainium2, SPMD over 8 NeuronCores.

Pipeline per core (core i owns mask rows 34i..34i+34, hm rows 17i..17i+17):
  - 3x3 conv (256->64) + relu on the out0 shard  (bf16 matmuls, col-tiled x2)
  - hm head (1x1 conv) + sigmoid on the out1 shard, 3x3 NMS, local top-8
  - allgather candidates (idx + pixel features), fp32 rescore, global top-4
  - params head evaluated only at the 4 winners (fp32)
  - dynamic heads (masks/regs) as a K=67 matmul over [conv|xx|yy|ones]
  - row-range MLP: local partial over h, AllReduce, relu, second matmul
Host: shards/pads inputs, reassembles full outputs.
"""
import sys
import numpy as np

sys.path.insert(0, "/opt/trn_rl_repo")

import ml_dtypes
import concourse.bass as bass
import concourse.bacc as bacc
import concourse.tile as tile
import concourse.mybir as mybir
from concourse.masks import make_identity

F32 = mybir.dt.float32
F32R = mybir.dt.float32r
BF16 = mybir.dt.bfloat16
U32 = mybir.dt.uint32
AF = mybir.ActivationFunctionType
OP = mybir.AluOpType

NCORE = 8
C = 256
HM, WM = 272, 480          # mask feature map
HH, WH = 136, 240          # hm feature map
RM, RH = HM // NCORE, HH // NCORE   # 34, 17 rows per core
PIX = RM * WM              # 16320
HPIX = RH * WH             # 4080
NMASK = 67                 # 66 weights + bias
NGEN = 134
K = 4                      # num_ins
NEG = -1.0e30

# matmul dtype for the hm head and the MLP (f32r = full-rate 4-byte mode)
HM_MODE = "f32r"
MLP_MODE = "f32r"


def _build():
    nc = bacc.Bacc("TRN2", target_bir_lowering=False, debug=False,
                   num_devices=NCORE)

    # ---------------- dram io ----------------
    x0_d = nc.dram_tensor("x0", [C, RM + 2, WM + 2], BF16, kind="ExternalInput")
    x1_d = nc.dram_tensor("x1", [C, RH + 2, WH], F32, kind="ExternalInput")
    x1p_d = nc.dram_tensor("x1p", [HPIX, C], F32, kind="ExternalInput")
    hbias_d = nc.dram_tensor("hbias", [RH + 2, 1], F32, kind="ExternalInput")
    ibase_d = nc.dram_tensor("ibase", [1, 1], F32, kind="ExternalInput")
    wt_d = nc.dram_tensor("wt", [128, 1152], BF16, kind="ExternalInput")
    mbb_d = nc.dram_tensor("mbb", [128, 1], F32, kind="ExternalInput")
    hmw_d = nc.dram_tensor("hmw", [128, 2], F32, kind="ExternalInput")
    pw_d = nc.dram_tensor("pw", [2, 128, NGEN], F32, kind="ExternalInput")
    pb_d = nc.dram_tensor("pb", [NGEN, 1], F32, kind="ExternalInput")
    coords_d = nc.dram_tensor("coords", [3, PIX], BF16, kind="ExternalInput")
    w1_d = nc.dram_tensor("w1loc", [RM, 64], F32, kind="ExternalInput")
    b1_d = nc.dram_tensor("b1", [64, 1], F32, kind="ExternalInput")
    w2_d = nc.dram_tensor("w2", [64, 2], F32, kind="ExternalInput")
    b2_d = nc.dram_tensor("b2", [2, 1], F32, kind="ExternalInput")

    mro_d = nc.dram_tensor("mro", [8, RM, WM], F32, kind="ExternalOutput")
    sco_d = nc.dram_tensor("sco", [1, K], F32, kind="ExternalOutput")
    ind_d = nc.dram_tensor("ind", [1, K], U32, kind="ExternalOutput")
    fro_d = nc.dram_tensor("fro", [K, WM, 2], F32, kind="ExternalOutput")

    hm_mm_dt = {"f32r": F32R, "f32": F32, "bf16": BF16}[HM_MODE]
    mlp_mm_dt = {"f32r": F32R, "f32": F32, "bf16": BF16}[MLP_MODE]
    hm_dma = nc.gpsimd if hm_mm_dt != F32 else nc.sync
    mlp_dma = nc.gpsimd if mlp_mm_dt != F32 else nc.sync

    with tile.TileContext(nc) as tc:
        with tc.tile_pool(name="big", bufs=2) as big, \
             tc.tile_pool(name="mid", bufs=2) as mid, \
             tc.tile_pool(name="x1pool", bufs=2) as x1pool, \
             tc.tile_pool(name="mr", bufs=2) as mrp, \
             tc.tile_pool(name="odd", bufs=2) as oddp, \
             tc.tile_pool(name="small", bufs=1) as sm, \
             tc.tile_pool(name="convps", bufs=3, space="PSUM") as cps, \
             tc.tile_pool(name="ps", bufs=2, space="PSUM") as sps, \
             tc.tile_pool(name="tp", bufs=2, space="PSUM") as tps, \
             tc.tile_pool(name="dram", bufs=1, space="DRAM") as dram:

            # ---------------- constant loads ----------------
            wt_sb = sm.tile([128, 1152], BF16, name="wt_sb")
            nc.sync.dma_start(wt_sb[:], wt_d.ap())
            mbb = sm.tile([128, 1], F32, name="mbb")
            nc.sync.dma_start(mbb[:], mbb_d.ap())
            hmw = sm.tile([128, 2], hm_mm_dt, name="hmw")
            hm_dma.dma_start(hmw[:], hmw_d.ap())
            pw0 = sm.tile([128, NGEN], F32, name="pw0")
            pw1 = sm.tile([128, NGEN], F32, name="pw1")
            nc.sync.dma_start(pw0[:], pw_d.ap()[0])
            nc.sync.dma_start(pw1[:], pw_d.ap()[1])
            pb0 = sm.tile([128, 1], F32, name="pb0")
            pb1 = sm.tile([6, 1], F32, name="pb1")
            nc.sync.dma_start(pb0[:], pb_d.ap()[0:128, :])
            nc.sync.dma_start(pb1[:], pb_d.ap()[128:134, :])
            w1loc = sm.tile([RM, 64], mlp_mm_dt, name="w1loc")
            mlp_dma.dma_start(w1loc[:], w1_d.ap())
            b1t = sm.tile([64, 1], F32, name="b1t")
            nc.sync.dma_start(b1t[:], b1_d.ap())
            w2t = sm.tile([64, 2], mlp_mm_dt, name="w2t")
            mlp_dma.dma_start(w2t[:], w2_d.ap())
            b2t = sm.tile([2, 1], F32, name="b2t")
            nc.sync.dma_start(b2t[:], b2_d.ap())
            hbias = sm.tile([RH + 2, 1], F32, name="hbias")
            nc.sync.dma_start(hbias[:], hbias_d.ap())
            ibase = sm.tile([1, 1], F32, name="ibase")
            nc.sync.dma_start(ibase[:], ibase_d.ap())
            ident = sm.tile([128, 128], F32, name="ident")
            make_identity(nc, ident[:])

            # x_cat: [0:64] conv+relu, 64 xx, 65 yy, 66 ones   (bf16)
            x_cat = sm.tile([NMASK, PIX], BF16, name="x_cat")
            nc.sync.dma_start(x_cat[64:67, :], coords_d.ap())

            # ---------------- hm head ----------------
            hm_flat = mid.tile([1, HPIX + 2 * WH], F32, name="hm_flat", tag="mid")
            row_groups = [(0, 10), (10, 19)]
            for (r0, r1) in row_groups:
                npx = (r1 - r0) * WH
                xg = [None, None]
                for ch in range(2):
                    xg[ch] = x1pool.tile([128, 2400], hm_mm_dt,
                                         name=f"x1g{ch}", tag="x1g")
                    hm_dma.dma_start(xg[ch][:, :npx],
                                     x1_d.ap()[128 * ch:128 * (ch + 1), r0:r1, :])
                off = 0
                while off < npx:
                    n = min(512, npx - off)
                    pshm = sps.tile([1, 512], F32, name="pshm", tag="ps", padded_shape=[128, 512])
                    for ch in range(2):
                        nc.tensor.matmul(pshm[:, :n], hmw[:, ch:ch + 1],
                                         xg[ch][:, off:off + n],
                                         start=(ch == 0), stop=(ch == 1))
                    nc.scalar.activation(hm_flat[0:1, r0 * WH + off:
                                                  r0 * WH + off + n],
                                         pshm[:, :n], AF.Sigmoid)
                    off += n

            # ---------------- NMS ----------------
            hm_pad = sm.tile([RH + 2, WH + 2], F32, name="hm_pad")
            nc.vector.memset(hm_pad[:], NEG)
            nc.sync.dma_start(hm_pad[:, 1:WH + 1], hm_flat[0:1, :])
            # invalidate out-of-image halo rows (bias 0 on valid rows)
            nc.vector.tensor_scalar(hm_pad[:], hm_pad[:], hbias[:, :], None,
                                    op0=OP.add)
            m1 = sm.tile([RH + 2, WH + 1], F32, name="m1")
            nc.vector.tensor_tensor(m1[:], hm_pad[:, 0:WH + 1],
                                    hm_pad[:, 1:WH + 2], op=OP.max)
            hmx = sm.tile([RH + 2, WH], F32, name="hmx")
            nc.vector.tensor_tensor(hmx[:], m1[:, 0:WH], m1[:, 1:WH + 1],
                                    op=OP.max)
            vB = sm.tile([RH, WH], F32, name="vB")
            vC = sm.tile([RH, WH], F32, name="vC")
            hmc = sm.tile([RH, WH], F32, name="hmc")
            nc.sync.dma_start(vB[:], hmx[1:RH + 1, :])
            nc.sync.dma_start(vC[:], hmx[2:RH + 2, :])
            nc.sync.dma_start(hmc[:], hm_pad[1:RH + 1, 1:WH + 1])
            vmax = sm.tile([RH, WH], F32, name="vmax")
            nc.vector.tensor_tensor(vmax[:], hmx[0:RH, :], vB[:], op=OP.max)
            nc.vector.tensor_tensor(vmax[:], vmax[:], vC[:], op=OP.max)
            kp = sm.tile([RH, WH], F32, name="kp")
            nc.vector.tensor_tensor(kp[:], vmax[:], hmc[:], op=OP.is_equal)
            heat = sm.tile([RH, WH], F32, name="heat")
            nc.vector.tensor_tensor(heat[:], kp[:], hmc[:], op=OP.mult)

            # ---------------- local top-8 ----------------
            flat = mid.tile([1, HPIX], F32, name="flat", tag="mid")
            nc.sync.dma_start(flat[0:1, :], heat[:])
            v8 = sm.tile([1, 8], F32, name="v8")
            i8 = sm.tile([1, 8], U32, name="i8")
            nc.vector.max_with_indices(v8[:], i8[:], flat[:])
            if8 = sm.tile([1, 8], F32, name="if8")
            nc.vector.tensor_copy(if8[:], i8[:])
            gidx8 = sm.tile([1, 8], F32, name="gidx8")
            nc.vector.tensor_scalar(gidx8[:], if8[:], ibase[:, :], None,
                                    op0=OP.add)
            i8p = sm.tile([8, 1], U32, name="i8p")
            nc.sync.dma_start(i8p[:, 0:1], i8[0:1, :])
            feat8 = sm.tile([8, C], F32, name="feat8")
            nc.gpsimd.indirect_dma_start(
                out=feat8[:], out_offset=None, in_=x1p_d.ap(),
                in_offset=bass.IndirectOffsetOnAxis(ap=i8p[:, :1], axis=0))

            # ---------------- allgather candidates ----------------
            agi_in = dram.tile([1, 8], F32, name="agi_in")
            agi_out = dram.tile([NCORE, 8], F32, name="agi_out",
                                addr_space="Shared")
            agf_in = dram.tile([8, C], F32, name="agf_in")
            agf_out = dram.tile([NCORE * 8, C], F32, name="agf_out",
                                addr_space="Shared")
            nc.gpsimd.dma_start(agi_in[:], gidx8[:])
            nc.gpsimd.dma_start(agf_in[:], feat8[:])
            nc.gpsimd.collective_compute(
                "AllGather", OP.bypass, replica_groups=[list(range(NCORE))],
                ins=[agi_in.opt()], outs=[agi_out.opt()])
            nc.gpsimd.collective_compute(
                "AllGather", OP.bypass, replica_groups=[list(range(NCORE))],
                ins=[agf_in.opt()], outs=[agf_out.opt()])

            # ---------------- fp32 rescore of all 64 candidates ----------------
            fa = sm.tile([64, C], F32, name="fa")
            nc.gpsimd.dma_start(fa[:], agf_out[:])
            featsT = sm.tile([128, 128], F32, name="featsT")
            for ch in range(2):
                pst = tps.tile([128, 64], F32, name="pst", tag="tp", padded_shape=[128, 512])
                nc.tensor.transpose(pst[:], fa[:, 128 * ch:128 * (ch + 1)],
                                    ident[0:64, 0:64])
                nc.scalar.copy(featsT[:, 64 * ch:64 * (ch + 1)], pst[:])
            psc = sps.tile([1, 64], F32, name="psc", tag="ps", padded_shape=[128, 512])
            hmw32 = sm.tile([128, 2], F32, name="hmw32")
            nc.sync.dma_start(hmw32[:], hmw_d.ap())
            for ch in range(2):
                nc.tensor.matmul(psc[:], hmw32[:, ch:ch + 1],
                                 featsT[:, 64 * ch:64 * (ch + 1)],
                                 start=(ch == 0), stop=(ch == 1))
            scoref = sm.tile([1, 64], F32, name="scoref")
            nc.scalar.activation(scoref[:], psc[:], AF.Sigmoid)

            # ---------------- global top-4 ----------------
            gv8 = sm.tile([1, 8], F32, name="gv8")
            gp8 = sm.tile([1, 8], U32, name="gp8")
            nc.vector.max_with_indices(gv8[:], gp8[:], scoref[:])
            nc.sync.dma_start(sco_d.ap(), gv8[0:1, 0:K])
            gp4 = sm.tile([K, 1], U32, name="gp4")
            nc.sync.dma_start(gp4[:, 0:1], gp8[0:1, 0:K])
            gi = sm.tile([K, 1], F32, name="gi")
            nc.gpsimd.indirect_dma_start(
                out=gi[:], out_offset=None,
                in_=agi_out.opt().rearrange("a b -> (a b)").unsqueeze(1),
                in_offset=bass.IndirectOffsetOnAxis(ap=gp4[:, :1], axis=0))
            giu = sm.tile([K, 1], U32, name="giu")
            nc.vector.tensor_copy(giu[:], gi[:])
            nc.sync.dma_start(ind_d.ap(), giu[:, 0:1])
            fw = sm.tile([K, C], F32, name="fw")
            nc.gpsimd.indirect_dma_start(
                out=fw[:], out_offset=None, in_=agf_out.opt(),
                in_offset=bass.IndirectOffsetOnAxis(ap=gp4[:, :1], axis=0))

            # ---------------- params head at the 4 winners ----------------
            fwT = sm.tile([128, 2 * K], F32, name="fwT")
            for ch in range(2):
                psf = tps.tile([128, K], F32, name="psf", tag="tp", padded_shape=[128, 512])
                nc.tensor.transpose(psf[:], fw[:, 128 * ch:128 * (ch + 1)],
                                    ident[0:K, 0:K])
                nc.scalar.copy(fwT[:, K * ch:K * (ch + 1)], psf[:])
            psa = sps.tile([128, K], F32, name="psa", tag="ps", padded_shape=[128, 512])
            psb = sps.tile([6, K], F32, name="psb", tag="ps", padded_shape=[128, 512])
            for ch, pwc in enumerate((pw0, pw1)):
                nc.tensor.matmul(psa[:], pwc[:, 0:128],
                                 fwT[:, K * ch:K * (ch + 1)],
                                 start=(ch == 0), stop=(ch == 1))
            for ch, pwc in enumerate((pw0, pw1)):
                nc.tensor.matmul(psb[:], pwc[:, 128:NGEN],
                                 fwT[:, K * ch:K * (ch + 1)],
                                 start=(ch == 0), stop=(ch == 1))
            sel_a = sm.tile([128, K], F32, name="sel_a")
            sel_b = sm.tile([6, K], F32, name="sel_b")
            nc.vector.tensor_scalar(sel_a[:], psa[:], pb0[:, :], None, op0=OP.add)
            nc.vector.tensor_scalar(sel_b[:], psb[:], pb1[:, :], None, op0=OP.add)

            # head weights [67, 8]: cols 0:4 masks (rows 0:67 of sel),
            # cols 4:8 regs (rows 67:134 of sel)
            hd = sm.tile([NMASK, 2 * K], BF16, name="hd")
            nc.vector.tensor_copy(hd[0:NMASK, 0:K], sel_a[0:NMASK, :])
            nc.gpsimd.dma_start(hd[0:61, K:2 * K], sel_a[67:128, :])
            nc.gpsimd.dma_start(hd[61:67, K:2 * K], sel_b[0:6, :])

            # ---------------- conv 3x3 + relu ----------------
            x0c = [None, None]
            for ch in range(2):
                x0c[ch] = big.tile([128, (RM + 2) * (WM + 2)], BF16,
                                   name=f"x0c{ch}", tag="big")
                nc.sync.dma_start(x0c[ch][:],
                                  x0_d.ap()[128 * ch:128 * (ch + 1), :, :])
            W2 = WM + 2
            for p in range(RM // 2):
                pcv = cps.tile([128, WM], F32, name="pcv", tag="cv", padded_shape=[128, 512])
                for si, (ch, dy, dx) in enumerate(
                        (c_, y_, x_) for c_ in range(2) for y_ in range(3)
                        for x_ in range(3)):
                    lhsT = wt_sb[:, (ch * 9 + dy * 3 + dx) * 64:
                                 (ch * 9 + dy * 3 + dx + 1) * 64]
                    ra = x0c[ch][:, (2 * p + dy) * W2 + dx:
                                 (2 * p + dy) * W2 + dx + WM]
                    rb = x0c[ch][:, (2 * p + 1 + dy) * W2 + dx:
                                 (2 * p + 1 + dy) * W2 + dx + WM]
                    nc.tensor.matmul(pcv[0:64, :], lhsT, ra,
                                     start=(si == 0), stop=(si == 17),
                                     skip_group_check=True)
                    nc.tensor.matmul(pcv[64:128, :], lhsT, rb,
                                     start=(si == 0), stop=(si == 17),
                                     skip_group_check=True)
                nc.scalar.activation(x_cat[0:64, (2 * p) * WM:(2 * p + 1) * WM],
                                     pcv[0:64, :], AF.Relu, bias=mbb[0:64, :])
                oddst = oddp.tile([128, WM], BF16, name="oddst", tag="odd")
                nc.scalar.activation(oddst[64:128, :], pcv[64:128, :],
                                     AF.Relu, bias=mbb[64:128, :])
                nc.sync.dma_start(
                    x_cat[0:64, (2 * p + 1) * WM:(2 * p + 2) * WM],
                    oddst[64:128, :])

            # ---------------- dynamic heads ----------------
            nchunks = (PIX + 511) // 512
            for half in range((nchunks + 1) // 2):
                mrb = mrp.tile([8, 1024], F32, name="mrb", tag="mr")
                w_here = 0
                for j in range(2):
                    ci = 2 * half + j
                    if ci >= nchunks:
                        break
                    n = min(512, PIX - 512 * ci)
                    psh = sps.tile([8, 512], F32, name="psh", tag="ps", padded_shape=[128, 512])
                    nc.tensor.matmul(psh[:, :n], hd[:, :],
                                     x_cat[:, 512 * ci:512 * ci + n],
                                     start=True, stop=True)
                    nc.scalar.copy(mrb[:, 512 * j:512 * j + n], psh[:, :n])
                    w_here += n
                nc.sync.dma_start(
                    mro_d.ap().rearrange("k r w -> k (r w)")
                    [:, 1024 * half:1024 * half + w_here],
                    mrb[:, :w_here])

            # ---------------- row-range MLP ----------------
            mt = big.tile([RM, 4 * WM], mlp_mm_dt, name="mt", tag="big")
            for k in range(K):
                mlp_dma.dma_start(mt[:, WM * k:WM * (k + 1)], mro_d.ap()[k])
            th = big.tile([128, 2048], F32, name="th", tag="big")
            h1 = th[0:64, 0:4 * WM]
            for k in range(K):
                psm = sps.tile([64, WM], F32, name="psm", tag="ps", padded_shape=[128, 512])
                nc.tensor.matmul(psm[:], w1loc[:, :], mt[:, WM * k:WM * (k + 1)],
                                 start=True, stop=True)
                nc.scalar.copy(h1[:, WM * k:WM * (k + 1)], psm[:])
            arh_in = dram.tile([64, 4 * WM], F32, name="arh_in")
            arh_out = dram.tile([64, 4 * WM], F32, name="arh_out",
                                addr_space="Shared")
            nc.gpsimd.dma_start(arh_in[:], h1)
            nc.gpsimd.collective_compute(
                "AllReduce", OP.add, replica_groups=[list(range(NCORE))],
                ins=[arh_in.opt()], outs=[arh_out.opt()])
            h1ar = th[64:128, 0:4 * WM]
            nc.gpsimd.dma_start(h1ar, arh_out[:])
            h1r = big.tile([64, 4 * WM], mlp_mm_dt, name="h1r", tag="big")
            nc.scalar.activation(h1r[:], h1ar, AF.Relu, bias=b1t[:, :])
            fr = mid.tile([2, 4 * WM], F32, name="fr", tag="mid")
            for k in range(K):
                psr = sps.tile([2, WM], F32, name="psr", tag="ps", padded_shape=[128, 512])
                nc.tensor.matmul(psr[:], w2t[:, :], h1r[:, WM * k:WM * (k + 1)],
                                 start=True, stop=True)
                nc.vector.tensor_scalar(fr[:, WM * k:WM * (k + 1)], psr[:],
                                        b2t[:, :], None, op0=OP.add)
            nc.sync.dma_start(fro_d.ap().rearrange("k w j -> j (k w)"), fr[:])

    nc.compile()
    return nc


_NC = None


def _get_nc():
    global _NC
    if _NC is None:
        _NC = _build()
    return _NC


def _prep_inputs(out0, out1, hm_w, hm_b, params_w, params_b, mb_w, mb_b,
                 mlp_w1, mlp_b1, mlp_w2, mlp_b2):
    bf = ml_dtypes.bfloat16
    o0p = np.zeros((C, HM + 2, WM + 2), dtype=bf)
    o0p[:, 1:HM + 1, 1:WM + 1] = out0[0].astype(bf)
    o1p = np.zeros((C, HH + 2, WH), dtype=np.float32)
    o1p[:, 1:HH + 1, :] = out1[0]
    x1p_full = np.ascontiguousarray(
        out1[0].transpose(1, 2, 0).reshape(HH * WH, C))

    wt = mb_w.transpose(1, 2, 3, 0).reshape(C, 9, 64)
    wt = wt.reshape(2, 128, 9, 64).transpose(1, 0, 2, 3).reshape(128, 1152)
    wt = np.ascontiguousarray(wt).astype(bf)
    mbb = np.tile(mb_b.reshape(64, 1), (2, 1)).astype(np.float32)
    hmw = np.ascontiguousarray(hm_w[0].reshape(2, 128).T).astype(np.float32)
    pw = np.ascontiguousarray(params_w.T.reshape(2, 128, NGEN)).astype(np.float32)
    pb = params_b.reshape(NGEN, 1).astype(np.float32)
    b1 = mlp_b1.reshape(64, 1).astype(np.float32)
    w2 = mlp_w2.astype(np.float32)
    b2 = mlp_b2.reshape(2, 1).astype(np.float32)

    xx = np.linspace(-1.0, 1.0, WM, dtype=np.float32)
    yy = np.linspace(-1.0, 1.0, HM, dtype=np.float32)
    ones = np.ones(PIX, dtype=np.float32)
    xxr = np.tile(xx, RM)

    in_maps = []
    for i in range(NCORE):
        hb = np.zeros((RH + 2, 1), np.float32)
        for r in range(RH + 2):
            g = RH * i - 1 + r
            if g < 0 or g >= HH:
                hb[r, 0] = NEG
        coords = np.stack([
            xxr,
            np.repeat(yy[RM * i:RM * (i + 1)], WM),
            ones]).astype(bf)
        in_maps.append({
            "x0": np.ascontiguousarray(o0p[:, RM * i:RM * i + RM + 2, :]),
            "x1": np.ascontiguousarray(o1p[:, RH * i:RH * i + RH + 2, :]),
            "x1p": np.ascontiguousarray(
                x1p_full[RH * WH * i:RH * WH * (i + 1)]),
            "hbias": hb,
            "ibase": np.array([[HPIX * i]], np.float32),
            "wt": wt, "mbb": mbb, "hmw": hmw, "pw": pw, "pb": pb,
            "coords": coords,
            "w1loc": np.ascontiguousarray(mlp_w1[RM * i:RM * (i + 1)]).astype(np.float32),
            "b1": b1, "w2": w2, "b2": b2,
        })
    return in_maps


def kernel(out0, out1, hm_w, hm_b, params_w, params_b, mb_w, mb_b,
           mlp_w1, mlp_b1, mlp_w2, mlp_b2, num_ins):
    assert int(num_ins) == K
    assert float(hm_b[0]) == 0.0
    nc = _get_nc()
    in_maps = _prep_inputs(out0, out1, hm_w, hm_b, params_w, params_b,
                           mb_w, mb_b, mlp_w1, mlp_b1, mlp_w2, mlp_b2)
    import os
    if os.environ.get("KERNEL_SIM"):
        from concourse.bass_interp import MultiCoreSim
        sim = MultiCoreSim(nc, num_cores=NCORE, trace=False,
                           require_finite=False, require_nnan=False)
        for c in range(NCORE):
            for name, arr in in_maps[c].items():
                sim.cores[c].tensor(name)[:] = arr
        sim.simulate(check_with_hw=False)
        res = [{nm: np.array(sim.cores[c].tensor(nm))
                for nm in ("mro", "sco", "ind", "fro")} for c in range(NCORE)]
    else:
        from concourse.bass_utils import run_bass_kernel_spmd
        res = run_bass_kernel_spmd(nc, in_maps, list(range(NCORE))).results

    masks0 = np.concatenate([res[i]["mro"][0:4] for i in range(NCORE)], axis=1)
    regs0 = np.concatenate([res[i]["mro"][4:8] for i in range(NCORE)], axis=1)
    scores = res[0]["sco"][0].astype(np.float32)
    inds = res[0]["ind"][0].astype(np.int32)
    feat_range = res[0]["fro"].astype(np.float32)
    return scores, inds, regs0, masks0, feat_range


# revision 8
# speedup vs baseline: 1.2135x; 1.2135x over previous
"""CondLaneNet head kernel for Trainium2, SPMD over 8 NeuronCores.

Pipeline per core (core i owns mask rows 34i..34i+34, hm rows 17i..17i+17):
  - 3x3 conv (256->64) + relu on the out0 shard  (bf16 matmuls, col-tiled x2)
  - hm head (1x1 conv) + sigmoid on the out1 shard, 3x3 NMS, local top-8
  - allgather candidates (idx + pixel features), fp32 rescore, global top-4
  - params head evaluated only at the 4 winners (fp32)
  - dynamic heads (masks/regs) as a K=67 matmul over [conv|xx|yy|ones]
  - row-range MLP: local partial over h, AllReduce, relu, second matmul
Host: shards/pads inputs, reassembles full outputs.
"""
import sys
import numpy as np

sys.path.insert(0, "/opt/trn_rl_repo")

import ml_dtypes
import concourse.bass as bass
import concourse.bacc as bacc
import concourse.tile as tile
import concourse.mybir as mybir
from concourse.masks import make_identity

F32 = mybir.dt.float32
F32R = mybir.dt.float32r
BF16 = mybir.dt.bfloat16
U32 = mybir.dt.uint32
AF = mybir.ActivationFunctionType
OP = mybir.AluOpType

NCORE = 8
C = 256
HM, WM = 272, 480          # mask feature map
HH, WH = 136, 240          # hm feature map
RM, RH = HM // NCORE, HH // NCORE   # 34, 17 rows per core
PIX = RM * WM              # 16320
HPIX = RH * WH             # 4080
NMASK = 67                 # 66 weights + bias
NGEN = 134
K = 4                      # num_ins
NEG = -1.0e30

# matmul dtype for the hm head and the MLP (f32r = full-rate 4-byte mode)
HM_MODE = "f32r"
MLP_MODE = "f32r"


def _build():
    nc = bacc.Bacc("TRN2", target_bir_lowering=False, debug=False,
                   num_devices=NCORE)

    # ---------------- dram io ----------------
    x0_d = nc.dram_tensor("x0", [C, RM + 2, WM + 2], BF16, kind="ExternalInput")
    x1_d = nc.dram_tensor("x1", [C, RH + 2, WH], F32, kind="ExternalInput")
    x1p_d = nc.dram_tensor("x1p", [HPIX, C], F32, kind="ExternalInput")
    hbias_d = nc.dram_tensor("hbias", [RH + 2, 1], F32, kind="ExternalInput")
    ibase_d = nc.dram_tensor("ibase", [1, 1], F32, kind="ExternalInput")
    wt_d = nc.dram_tensor("wt", [128, 1152], BF16, kind="ExternalInput")
    mbb_d = nc.dram_tensor("mbb", [128, 1], F32, kind="ExternalInput")
    hmw_d = nc.dram_tensor("hmw", [128, 2], F32, kind="ExternalInput")
    pw_d = nc.dram_tensor("pw", [2, 128, NGEN], F32, kind="ExternalInput")
    pb_d = nc.dram_tensor("pb", [NGEN, 1], F32, kind="ExternalInput")
    coords_d = nc.dram_tensor("coords", [3, PIX], BF16, kind="ExternalInput")
    w1_d = nc.dram_tensor("w1loc", [RM, 64], F32, kind="ExternalInput")
    b1_d = nc.dram_tensor("b1", [64, 1], F32, kind="ExternalInput")
    w2_d = nc.dram_tensor("w2", [64, 2], F32, kind="ExternalInput")
    b2_d = nc.dram_tensor("b2", [2, 1], F32, kind="ExternalInput")

    mro_d = nc.dram_tensor("mro", [8, RM, WM], F32, kind="ExternalOutput")
    sco_d = nc.dram_tensor("sco", [1, K], F32, kind="ExternalOutput")
    ind_d = nc.dram_tensor("ind", [1, K], U32, kind="ExternalOutput")
    fro_d = nc.dram_tensor("fro", [2, K * WM], F32, kind="ExternalOutput")

    hm_mm_dt = {"f32r": F32R, "f32": F32, "bf16": BF16}[HM_MODE]
    mlp_mm_dt = {"f32r": F32R, "f32": F32, "bf16": BF16}[MLP_MODE]
    hm_dma = nc.gpsimd if hm_mm_dt != F32 else nc.sync
    mlp_dma = nc.gpsimd if mlp_mm_dt != F32 else nc.sync

    with tile.TileContext(nc) as tc:
        with tc.tile_pool(name="big", bufs=2) as big, \
             tc.tile_pool(name="mid", bufs=2) as mid, \
             tc.tile_pool(name="x1pool", bufs=2) as x1pool, \
             tc.tile_pool(name="mr", bufs=2) as mrp, \
             tc.tile_pool(name="odd", bufs=2) as oddp, \
             tc.tile_pool(name="small", bufs=1) as sm, \
             tc.tile_pool(name="convps", bufs=3, space="PSUM") as cps, \
             tc.tile_pool(name="ps", bufs=3, space="PSUM") as sps, \
             tc.tile_pool(name="tp", bufs=2, space="PSUM") as tps, \
             tc.tile_pool(name="dram", bufs=1, space="DRAM") as dram:

            # ---------------- constant loads ----------------
            wt_sb = sm.tile([128, 1152], BF16, name="wt_sb")
            nc.sync.dma_start(wt_sb[:], wt_d.ap())
            x0c = [None, None]
            for ch in range(2):
                x0c[ch] = big.tile([128, (RM + 2) * (WM + 2)], BF16,
                                   name=f"x0c{ch}", tag="big")
                nc.sync.dma_start(x0c[ch][:],
                                  x0_d.ap()[128 * ch:128 * (ch + 1), :, :])
            mbb = sm.tile([128, 1], F32, name="mbb")
            nc.sync.dma_start(mbb[:], mbb_d.ap())
            hmw = sm.tile([128, 2], hm_mm_dt, name="hmw")
            hm_dma.dma_start(hmw[:], hmw_d.ap())
            pw0 = sm.tile([128, NGEN], F32, name="pw0")
            pw1 = sm.tile([128, NGEN], F32, name="pw1")
            nc.sync.dma_start(pw0[:], pw_d.ap()[0])
            nc.sync.dma_start(pw1[:], pw_d.ap()[1])
            pb0 = sm.tile([128, 1], F32, name="pb0")
            pb1 = sm.tile([6, 1], F32, name="pb1")
            nc.sync.dma_start(pb0[:], pb_d.ap()[0:128, :])
            nc.sync.dma_start(pb1[:], pb_d.ap()[128:134, :])
            w1loc = sm.tile([RM, 64], mlp_mm_dt, name="w1loc")
            mlp_dma.dma_start(w1loc[:], w1_d.ap())
            b1t = sm.tile([64, 1], F32, name="b1t")
            nc.sync.dma_start(b1t[:], b1_d.ap())
            w2t = sm.tile([64, 2], mlp_mm_dt, name="w2t")
            mlp_dma.dma_start(w2t[:], w2_d.ap())
            b2t = sm.tile([2, 1], F32, name="b2t")
            nc.sync.dma_start(b2t[:], b2_d.ap())
            hbias = sm.tile([RH + 2, 1], F32, name="hbias")
            nc.sync.dma_start(hbias[:], hbias_d.ap())
            ibase = sm.tile([1, 1], F32, name="ibase")
            nc.sync.dma_start(ibase[:], ibase_d.ap())
            ident = sm.tile([128, 128], F32, name="ident")
            make_identity(nc, ident[:])

            # x_cat: [0:64] conv+relu, 64 xx, 65 yy, 66 ones   (bf16)
            x_cat = sm.tile([NMASK, PIX], BF16, name="x_cat")
            nc.sync.dma_start(x_cat[64:67, :], coords_d.ap())

            # ---------------- hm head ----------------
            hm_flat = mid.tile([1, HPIX + 2 * WH], F32, name="hm_flat", tag="mid")
            row_groups = [(0, 10), (10, 19)]
            for (r0, r1) in row_groups:
                npx = (r1 - r0) * WH
                xg = [None, None]
                for ch in range(2):
                    xg[ch] = x1pool.tile([128, 2400], hm_mm_dt,
                                         name=f"x1g{ch}", tag="x1g")
                    hm_dma.dma_start(xg[ch][:, :npx],
                                     x1_d.ap()[128 * ch:128 * (ch + 1), r0:r1, :])
                off = 0
                while off < npx:
                    n = min(512, npx - off)
                    pshm = sps.tile([1, 512], F32, name="pshm", tag="ps", padded_shape=[128, 512])
                    for ch in range(2):
                        nc.tensor.matmul(pshm[:, :n], hmw[:, ch:ch + 1],
                                         xg[ch][:, off:off + n],
                                         start=(ch == 0), stop=(ch == 1))
                    nc.scalar.activation(hm_flat[0:1, r0 * WH + off:
                                                  r0 * WH + off + n],
                                         pshm[:, :n], AF.Sigmoid)
                    off += n

            # ---------------- NMS ----------------
            hm_pad = sm.tile([RH + 2, WH + 2], F32, name="hm_pad")
            nc.vector.memset(hm_pad[:], NEG)
            nc.gpsimd.dma_start(hm_pad[:, 1:WH + 1], hm_flat[0:1, :])
            # invalidate out-of-image halo rows (bias 0 on valid rows)
            nc.vector.tensor_scalar(hm_pad[:], hm_pad[:], hbias[:, :], None,
                                    op0=OP.add)
            m1 = sm.tile([RH + 2, WH + 1], F32, name="m1")
            nc.vector.tensor_tensor(m1[:], hm_pad[:, 0:WH + 1],
                                    hm_pad[:, 1:WH + 2], op=OP.max)
            hmx = sm.tile([RH + 2, WH], F32, name="hmx")
            nc.vector.tensor_tensor(hmx[:], m1[:, 0:WH], m1[:, 1:WH + 1],
                                    op=OP.max)
            vB = sm.tile([RH, WH], F32, name="vB")
            vC = sm.tile([RH, WH], F32, name="vC")
            hmc = sm.tile([RH, WH], F32, name="hmc")
            nc.gpsimd.dma_start(vB[:], hmx[1:RH + 1, :])
            nc.gpsimd.dma_start(vC[:], hmx[2:RH + 2, :])
            nc.gpsimd.dma_start(hmc[:], hm_pad[1:RH + 1, 1:WH + 1])
            vmax = sm.tile([RH, WH], F32, name="vmax")
            nc.vector.tensor_tensor(vmax[:], hmx[0:RH, :], vB[:], op=OP.max)
            nc.vector.tensor_tensor(vmax[:], vmax[:], vC[:], op=OP.max)
            kp = sm.tile([RH, WH], F32, name="kp")
            nc.vector.tensor_tensor(kp[:], vmax[:], hmc[:], op=OP.is_equal)
            heat = sm.tile([RH, WH], F32, name="heat")
            nc.vector.tensor_tensor(heat[:], kp[:], hmc[:], op=OP.mult)

            # ---------------- local top-8 ----------------
            flat = mid.tile([1, HPIX], F32, name="flat", tag="mid")
            nc.gpsimd.dma_start(flat[0:1, :], heat[:])
            v8 = sm.tile([1, 8], F32, name="v8")
            i8 = sm.tile([1, 8], U32, name="i8")
            nc.vector.max_with_indices(v8[:], i8[:], flat[:])
            if8 = sm.tile([1, 8], F32, name="if8")
            nc.vector.tensor_copy(if8[:], i8[:])
            gidx8 = sm.tile([1, 8], F32, name="gidx8")
            nc.vector.tensor_scalar(gidx8[:], if8[:], ibase[:, :], None,
                                    op0=OP.add)
            i8p = sm.tile([8, 1], U32, name="i8p")
            nc.gpsimd.dma_start(i8p[:, 0:1], i8[0:1, :])
            feat8 = sm.tile([8, C], F32, name="feat8")
            nc.gpsimd.indirect_dma_start(
                out=feat8[:], out_offset=None, in_=x1p_d.ap(),
                in_offset=bass.IndirectOffsetOnAxis(ap=i8p[:, :1], axis=0))

            # ---------------- allgather candidates ----------------
            agi_in = dram.tile([1, 8], F32, name="agi_in")
            agi_out = dram.tile([NCORE, 8], F32, name="agi_out",
                                addr_space="Shared")
            agf_in = dram.tile([8, C], F32, name="agf_in")
            agf_out = dram.tile([NCORE * 8, C], F32, name="agf_out",
                                addr_space="Shared")
            nc.gpsimd.dma_start(agi_in[:], gidx8[:])
            nc.gpsimd.dma_start(agf_in[:], feat8[:])
            nc.gpsimd.collective_compute(
                "AllGather", OP.bypass, replica_groups=[list(range(NCORE))],
                ins=[agi_in.opt()], outs=[agi_out.opt()])
            nc.gpsimd.collective_compute(
                "AllGather", OP.bypass, replica_groups=[list(range(NCORE))],
                ins=[agf_in.opt()], outs=[agf_out.opt()])

            # ---------------- fp32 rescore of all 64 candidates ----------------
            fa = sm.tile([64, C], F32, name="fa")
            nc.gpsimd.dma_start(fa[:], agf_out[:])
            featsT = sm.tile([128, 128], F32, name="featsT")
            for ch in range(2):
                pst = tps.tile([128, 64], F32, name="pst", tag="tp", padded_shape=[128, 512])
                nc.tensor.transpose(pst[:], fa[:, 128 * ch:128 * (ch + 1)],
                                    ident[0:64, 0:64])
                nc.scalar.copy(featsT[:, 64 * ch:64 * (ch + 1)], pst[:])
            psc = sps.tile([1, 64], F32, name="psc", tag="ps", padded_shape=[128, 512])
            hmw32 = sm.tile([128, 2], F32, name="hmw32")
            nc.gpsimd.dma_start(hmw32[:], hmw_d.ap())
            for ch in range(2):
                nc.tensor.matmul(psc[:], hmw32[:, ch:ch + 1],
                                 featsT[:, 64 * ch:64 * (ch + 1)],
                                 start=(ch == 0), stop=(ch == 1))
            scoref = sm.tile([1, 64], F32, name="scoref")
            nc.scalar.activation(scoref[:], psc[:], AF.Sigmoid)

            # ---------------- global top-4 ----------------
            gv8 = sm.tile([1, 8], F32, name="gv8")
            gp8 = sm.tile([1, 8], U32, name="gp8")
            nc.vector.max_with_indices(gv8[:], gp8[:], scoref[:])
            nc.sync.dma_start(sco_d.ap(), gv8[0:1, 0:K])
            gp4 = sm.tile([K, 1], U32, name="gp4")
            nc.gpsimd.dma_start(gp4[:, 0:1], gp8[0:1, 0:K])
            gi = sm.tile([K, 1], F32, name="gi")
            nc.gpsimd.indirect_dma_start(
                out=gi[:], out_offset=None,
                in_=agi_out.opt().rearrange("a b -> (a b)").unsqueeze(1),
                in_offset=bass.IndirectOffsetOnAxis(ap=gp4[:, :1], axis=0))
            giu = sm.tile([K, 1], U32, name="giu")
            nc.vector.tensor_copy(giu[:], gi[:])
            nc.sync.dma_start(ind_d.ap(), giu[:, 0:1])
            fw = sm.tile([K, C], F32, name="fw")
            nc.gpsimd.indirect_dma_start(
                out=fw[:], out_offset=None, in_=agf_out.opt(),
                in_offset=bass.IndirectOffsetOnAxis(ap=gp4[:, :1], axis=0))

            # ---------------- params head at the 4 winners ----------------
            fwT = sm.tile([128, 2 * K], F32, name="fwT")
            for ch in range(2):
                psf = tps.tile([128, K], F32, name="psf", tag="tp", padded_shape=[128, 512])
                nc.tensor.transpose(psf[:], fw[:, 128 * ch:128 * (ch + 1)],
                                    ident[0:K, 0:K])
                nc.scalar.copy(fwT[:, K * ch:K * (ch + 1)], psf[:])
            psa = sps.tile([128, K], F32, name="psa", tag="ps", padded_shape=[128, 512])
            psb = sps.tile([6, K], F32, name="psb", tag="ps", padded_shape=[128, 512])
            for ch, pwc in enumerate((pw0, pw1)):
                nc.tensor.matmul(psa[:], pwc[:, 0:128],
                                 fwT[:, K * ch:K * (ch + 1)],
                                 start=(ch == 0), stop=(ch == 1))
            for ch, pwc in enumerate((pw0, pw1)):
                nc.tensor.matmul(psb[:], pwc[:, 128:NGEN],
                                 fwT[:, K * ch:K * (ch + 1)],
                                 start=(ch == 0), stop=(ch == 1))
            sel_a = sm.tile([128, K], F32, name="sel_a")
            sel_b = sm.tile([6, K], F32, name="sel_b")
            nc.vector.tensor_scalar(sel_a[:], psa[:], pb0[:, :], None, op0=OP.add)
            nc.vector.tensor_scalar(sel_b[:], psb[:], pb1[:, :], None, op0=OP.add)

            # head weights [67, 8]: cols 0:4 masks (rows 0:67 of sel),
            # cols 4:8 regs (rows 67:134 of sel)
            hd = sm.tile([NMASK, 2 * K], BF16, name="hd")
            nc.vector.tensor_copy(hd[0:NMASK, 0:K], sel_a[0:NMASK, :])
            nc.gpsimd.dma_start(hd[0:61, K:2 * K], sel_a[67:128, :])
            nc.gpsimd.dma_start(hd[61:67, K:2 * K], sel_b[0:6, :])

            # ---------------- conv 3x3 + relu ----------------
            W2 = WM + 2
            for p in range(RM // 2):
                pcv = cps.tile([128, WM], F32, name="pcv", tag="cv", padded_shape=[128, 512])
                for si, (ch, dy, dx) in enumerate(
                        (c_, y_, x_) for c_ in range(2) for y_ in range(3)
                        for x_ in range(3)):
                    lhsT = wt_sb[:, (ch * 9 + dy * 3 + dx) * 64:
                                 (ch * 9 + dy * 3 + dx + 1) * 64]
                    ra = x0c[ch][:, (2 * p + dy) * W2 + dx:
                                 (2 * p + dy) * W2 + dx + WM]
                    rb = x0c[ch][:, (2 * p + 1 + dy) * W2 + dx:
                                 (2 * p + 1 + dy) * W2 + dx + WM]
                    nc.tensor.matmul(pcv[0:64, :], lhsT, ra,
                                     start=(si == 0), stop=(si == 17),
                                     skip_group_check=True)
                    nc.tensor.matmul(pcv[64:128, :], lhsT, rb,
                                     start=(si == 0), stop=(si == 17),
                                     skip_group_check=True)
                nc.scalar.activation(x_cat[0:64, (2 * p) * WM:(2 * p + 1) * WM],
                                     pcv[0:64, :], AF.Relu, bias=mbb[0:64, :])
                oddst = oddp.tile([128, WM], BF16, name="oddst", tag="odd")
                nc.scalar.activation(oddst[64:128, :], pcv[64:128, :],
                                     AF.Relu, bias=mbb[64:128, :])
                nc.sync.dma_start(
                    x_cat[0:64, (2 * p + 1) * WM:(2 * p + 2) * WM],
                    oddst[64:128, :])

            # ---------------- dynamic heads ----------------
            for half in range(RM // 2):
                mrb = mrp.tile([8, 2 * WM], F32, name="mrb", tag="mr")
                for j in range(2):
                    r = 2 * half + j
                    psh = sps.tile([8, WM], F32, name="psh", tag="ps",
                                   padded_shape=[128, 512])
                    nc.tensor.matmul(psh[:], hd[:, :],
                                     x_cat[:, WM * r:WM * (r + 1)],
                                     start=True, stop=True)
                    if j == 0:
                        nc.scalar.copy(mrb[:, :WM], psh[:])
                    else:
                        nc.vector.tensor_copy(mrb[:, WM:], psh[:])
                nc.sync.dma_start(
                    mro_d.ap().rearrange("k r w -> k (r w)")
                    [:, 2 * WM * half:2 * WM * (half + 1)],
                    mrb[:])

            # ---------------- row-range MLP ----------------
            mt = big.tile([RM, 4 * WM], mlp_mm_dt, name="mt", tag="big")
            for k in range(K):
                mlp_dma.dma_start(mt[:, WM * k:WM * (k + 1)], mro_d.ap()[k])
            th = big.tile([128, 2048], F32, name="th", tag="big")
            h1 = th[0:64, 0:4 * WM]
            for k in range(K):
                psm = sps.tile([64, WM], F32, name="psm", tag="ps", padded_shape=[128, 512])
                nc.tensor.matmul(psm[:], w1loc[:, :], mt[:, WM * k:WM * (k + 1)],
                                 start=True, stop=True)
                nc.scalar.copy(h1[:, WM * k:WM * (k + 1)], psm[:])
            arh_in = dram.tile([64, 4 * WM], F32, name="arh_in")
            arh_out = dram.tile([64, 4 * WM], F32, name="arh_out",
                                addr_space="Shared")
            nc.gpsimd.dma_start(arh_in[:], h1)
            nc.gpsimd.collective_compute(
                "AllReduce", OP.add, replica_groups=[list(range(NCORE))],
                ins=[arh_in.opt()], outs=[arh_out.opt()])
            h1ar = th[64:128, 0:4 * WM]
            nc.gpsimd.dma_start(h1ar, arh_out[:])
            h1r = big.tile([64, 4 * WM], mlp_mm_dt, name="h1r", tag="big")
            nc.scalar.activation(h1r[:], h1ar, AF.Relu, bias=b1t[:, :])
            fr = mid.tile([2, 4 * WM], F32, name="fr", tag="mid")
            for k in range(K):
                psr = sps.tile([2, WM], F32, name="psr", tag="ps", padded_shape=[128, 512])
                nc.tensor.matmul(psr[:], w2t[:, :], h1r[:, WM * k:WM * (k + 1)],
                                 start=True, stop=True)
                nc.vector.tensor_scalar(fr[:, WM * k:WM * (k + 1)], psr[:],
                                        b2t[:, :], None, op0=OP.add)
            nc.sync.dma_start(fro_d.ap(), fr[:])

    nc.compile()
    return nc


_NC = None


def _get_nc():
    global _NC
    if _NC is None:
        _NC = _build()
    return _NC


def _prep_inputs(out0, out1, hm_w, hm_b, params_w, params_b, mb_w, mb_b,
                 mlp_w1, mlp_b1, mlp_w2, mlp_b2):
    bf = ml_dtypes.bfloat16
    o0p = np.zeros((C, HM + 2, WM + 2), dtype=bf)
    o0p[:, 1:HM + 1, 1:WM + 1] = out0[0].astype(bf)
    o1p = np.zeros((C, HH + 2, WH), dtype=np.float32)
    o1p[:, 1:HH + 1, :] = out1[0]
    x1p_full = np.ascontiguousarray(
        out1[0].transpose(1, 2, 0).reshape(HH * WH, C))

    wt = mb_w.transpose(1, 2, 3, 0).reshape(C, 9, 64)
    wt = wt.reshape(2, 128, 9, 64).transpose(1, 0, 2, 3).reshape(128, 1152)
    wt = np.ascontiguousarray(wt).astype(bf)
    mbb = np.tile(mb_b.reshape(64, 1), (2, 1)).astype(np.float32)
    hmw = np.ascontiguousarray(hm_w[0].reshape(2, 128).T).astype(np.float32)
    pw = np.ascontiguousarray(params_w.T.reshape(2, 128, NGEN)).astype(np.float32)
    pb = params_b.reshape(NGEN, 1).astype(np.float32)
    b1 = mlp_b1.reshape(64, 1).astype(np.float32)
    w2 = mlp_w2.astype(np.float32)
    b2 = mlp_b2.reshape(2, 1).astype(np.float32)

    xx = np.linspace(-1.0, 1.0, WM, dtype=np.float32)
    yy = np.linspace(-1.0, 1.0, HM, dtype=np.float32)
    ones = np.ones(PIX, dtype=np.float32)
    xxr = np.tile(xx, RM)

    in_maps = []
    for i in range(NCORE):
        hb = np.zeros((RH + 2, 1), np.float32)
        for r in range(RH + 2):
            g = RH * i - 1 + r
            if g < 0 or g >= HH:
                hb[r, 0] = NEG
        coords = np.stack([
            xxr,
            np.repeat(yy[RM * i:RM * (i + 1)], WM),
            ones]).astype(bf)
        in_maps.append({
            "x0": np.ascontiguousarray(o0p[:, RM * i:RM * i + RM + 2, :]),
            "x1": np.ascontiguousarray(o1p[:, RH * i:RH * i + RH + 2, :]),
            "x1p": np.ascontiguousarray(
                x1p_full[RH * WH * i:RH * WH * (i + 1)]),
            "hbias": hb,
            "ibase": np.array([[HPIX * i]], np.float32),
            "wt": wt, "mbb": mbb, "hmw": hmw, "pw": pw, "pb": pb,
            "coords": coords,
            "w1loc": np.ascontiguousarray(mlp_w1[RM * i:RM * (i + 1)]).astype(np.float32),
            "b1": b1, "w2": w2, "b2": b2,
        })
    return in_maps


def kernel(out0, out1, hm_w, hm_b, params_w, params_b, mb_w, mb_b,
           mlp_w1, mlp_b1, mlp_w2, mlp_b2, num_ins):
    assert int(num_ins) == K
    assert float(hm_b[0]) == 0.0
    nc = _get_nc()
    in_maps = _prep_inputs(out0, out1, hm_w, hm_b, params_w, params_b,
                           mb_w, mb_b, mlp_w1, mlp_b1, mlp_w2, mlp_b2)
    import os
    if os.environ.get("KERNEL_SIM"):
        from concourse.bass_interp import MultiCoreSim
        sim = MultiCoreSim(nc, num_cores=NCORE, trace=False,
                           require_finite=False, require_nnan=False)
        for c in range(NCORE):
            for name, arr in in_maps[c].items():
                sim.cores[c].tensor(name)[:] = arr
        sim.simulate(check_with_hw=False)
        res = [{nm: np.array(sim.cores[c].tensor(nm))
                for nm in ("mro", "sco", "ind", "fro")} for c in range(NCORE)]
    else:
        from concourse.bass_utils import run_bass_kernel_spmd
        res = run_bass_kernel_spmd(nc, in_maps, list(range(NCORE))).results

    masks0 = np.concatenate([res[i]["mro"][0:4] for i in range(NCORE)], axis=1)
    regs0 = np.concatenate([res[i]["mro"][4:8] for i in range(NCORE)], axis=1)
    scores = res[0]["sco"][0].astype(np.float32)
    inds = res[0]["ind"][0].astype(np.int32)
    feat_range = np.ascontiguousarray(
        res[0]["fro"].reshape(2, K, WM).transpose(1, 2, 0)).astype(np.float32)
    return scores, inds, regs0, masks0, feat_range


# revision 9
# speedup vs baseline: 1.2804x; 1.0551x over previous
"""CondLaneNet head kernel for Trainium2, SPMD over 8 NeuronCores.

Pipeline per core (core i owns mask rows 34i..34i+34, hm rows 17i..17i+17):
  - 3x3 conv (256->64) + relu on the out0 shard  (bf16 matmuls, col-tiled x2)
  - hm head (1x1 conv) + sigmoid on the out1 shard, 3x3 NMS, local top-8
  - allgather candidates (idx + pixel features), fp32 rescore, global top-4
  - params head evaluated only at the 4 winners (fp32)
  - dynamic heads (masks/regs) as a K=67 matmul over [conv|xx|yy|ones]
  - row-range MLP: local partial over h, AllReduce, relu, second matmul
Host: shards/pads inputs, reassembles full outputs.
"""
import sys
import numpy as np

sys.path.insert(0, "/opt/trn_rl_repo")

import ml_dtypes
import concourse.bass as bass
import concourse.bacc as bacc
import concourse.tile as tile
import concourse.mybir as mybir
from concourse.masks import make_identity

F32 = mybir.dt.float32
F32R = mybir.dt.float32r
BF16 = mybir.dt.bfloat16
U32 = mybir.dt.uint32
AF = mybir.ActivationFunctionType
OP = mybir.AluOpType

NCORE = 8
C = 256
HM, WM = 272, 480          # mask feature map
HH, WH = 136, 240          # hm feature map
RM, RH = HM // NCORE, HH // NCORE   # 34, 17 rows per core
PIX = RM * WM              # 16320
HPIX = RH * WH             # 4080
NMASK = 67                 # 66 weights + bias
NGEN = 134
K = 4                      # num_ins
NEG = -1.0e30

# matmul dtype for the hm head and the MLP (f32r = full-rate 4-byte mode)
HM_MODE = "f32r"
MLP_MODE = "f32r"


def _build():
    nc = bacc.Bacc("TRN2", target_bir_lowering=False, debug=False,
                   num_devices=NCORE)

    # ---------------- dram io ----------------
    x0_d = nc.dram_tensor("x0", [C, RM + 2, WM + 2], BF16, kind="ExternalInput")
    x1_d = nc.dram_tensor("x1", [C, RH + 2, WH], F32, kind="ExternalInput")
    x1p_d = nc.dram_tensor("x1p", [HPIX, C], F32, kind="ExternalInput")
    hbias_d = nc.dram_tensor("hbias", [RH + 2, 1], F32, kind="ExternalInput")
    ibase_d = nc.dram_tensor("ibase", [1, 1], F32, kind="ExternalInput")
    wt_d = nc.dram_tensor("wt", [128, 1152], BF16, kind="ExternalInput")
    mbb_d = nc.dram_tensor("mbb", [128, 1], F32, kind="ExternalInput")
    hmw_d = nc.dram_tensor("hmw", [128, 2], F32, kind="ExternalInput")
    pw_d = nc.dram_tensor("pw", [2, 128, NGEN], F32, kind="ExternalInput")
    pb_d = nc.dram_tensor("pb", [NGEN, 1], F32, kind="ExternalInput")
    coords_d = nc.dram_tensor("coords", [3, PIX], BF16, kind="ExternalInput")
    w1_d = nc.dram_tensor("w1loc", [RM, 64], F32, kind="ExternalInput")
    b1_d = nc.dram_tensor("b1", [64, 1], F32, kind="ExternalInput")
    w2_d = nc.dram_tensor("w2", [64, 2], F32, kind="ExternalInput")
    b2_d = nc.dram_tensor("b2", [2, 1], F32, kind="ExternalInput")

    mro_d = nc.dram_tensor("mro", [8, RM, WM], F32, kind="ExternalOutput")
    sco_d = nc.dram_tensor("sco", [1, K], F32, kind="ExternalOutput")
    ind_d = nc.dram_tensor("ind", [1, K], U32, kind="ExternalOutput")
    fro_d = nc.dram_tensor("fro", [2, K * WM], F32, kind="ExternalOutput")

    hm_mm_dt = {"f32r": F32R, "f32": F32, "bf16": BF16}[HM_MODE]
    mlp_mm_dt = {"f32r": F32R, "f32": F32, "bf16": BF16}[MLP_MODE]
    hm_dma = nc.gpsimd if hm_mm_dt != F32 else nc.sync
    mlp_dma = nc.gpsimd if mlp_mm_dt != F32 else nc.sync

    with tile.TileContext(nc) as tc:
        with tc.tile_pool(name="big", bufs=2) as big, \
             tc.tile_pool(name="mid", bufs=2) as mid, \
             tc.tile_pool(name="x1pool", bufs=2) as x1pool, \
             tc.tile_pool(name="mr", bufs=2) as mrp, \
             tc.tile_pool(name="odd", bufs=3) as oddp, \
             tc.tile_pool(name="small", bufs=1) as sm, \
             tc.tile_pool(name="convps", bufs=4, space="PSUM") as cps, \
             tc.tile_pool(name="ps", bufs=3, space="PSUM") as sps, \
             tc.tile_pool(name="tp", bufs=1, space="PSUM") as tps, \
             tc.tile_pool(name="dram", bufs=1, space="DRAM") as dram:

            # ---------------- constant loads ----------------
            wt_sb = sm.tile([128, 1152], BF16, name="wt_sb")
            nc.sync.dma_start(wt_sb[:], wt_d.ap())
            x0c = [None, None]
            for ch in range(2):
                x0c[ch] = big.tile([128, (RM + 2) * (WM + 2)], BF16,
                                   name=f"x0c{ch}", tag="big")
                nc.sync.dma_start(x0c[ch][:],
                                  x0_d.ap()[128 * ch:128 * (ch + 1), :, :])
            mbb = sm.tile([128, 1], F32, name="mbb")
            nc.sync.dma_start(mbb[:], mbb_d.ap())
            hmw = sm.tile([128, 2], hm_mm_dt, name="hmw")
            hm_dma.dma_start(hmw[:], hmw_d.ap())
            pw0 = sm.tile([128, NGEN], F32, name="pw0")
            pw1 = sm.tile([128, NGEN], F32, name="pw1")
            nc.sync.dma_start(pw0[:], pw_d.ap()[0])
            nc.sync.dma_start(pw1[:], pw_d.ap()[1])
            pb0 = sm.tile([128, 1], F32, name="pb0")
            pb1 = sm.tile([6, 1], F32, name="pb1")
            nc.sync.dma_start(pb0[:], pb_d.ap()[0:128, :])
            nc.sync.dma_start(pb1[:], pb_d.ap()[128:134, :])
            w1loc = sm.tile([RM, 64], mlp_mm_dt, name="w1loc")
            mlp_dma.dma_start(w1loc[:], w1_d.ap())
            b1t = sm.tile([64, 1], F32, name="b1t")
            nc.sync.dma_start(b1t[:], b1_d.ap())
            w2t = sm.tile([64, 2], mlp_mm_dt, name="w2t")
            mlp_dma.dma_start(w2t[:], w2_d.ap())
            b2t = sm.tile([2, 1], F32, name="b2t")
            nc.sync.dma_start(b2t[:], b2_d.ap())
            hbias = sm.tile([RH + 2, 1], F32, name="hbias")
            nc.sync.dma_start(hbias[:], hbias_d.ap())
            ibase = sm.tile([1, 1], F32, name="ibase")
            nc.sync.dma_start(ibase[:], ibase_d.ap())
            ident = sm.tile([128, 128], F32, name="ident")
            make_identity(nc, ident[:])

            # x_cat: [0:64] conv+relu, 64 xx, 65 yy, 66 ones   (bf16)
            x_cat = sm.tile([NMASK, PIX], BF16, name="x_cat")
            nc.sync.dma_start(x_cat[64:67, :], coords_d.ap())

            # ---------------- hm head ----------------
            hm_flat = mid.tile([1, HPIX + 2 * WH], F32, name="hm_flat", tag="mid")
            row_groups = [(0, 10), (10, 19)]
            for (r0, r1) in row_groups:
                npx = (r1 - r0) * WH
                xg = [None, None]
                for ch in range(2):
                    xg[ch] = x1pool.tile([128, 2400], hm_mm_dt,
                                         name=f"x1g{ch}", tag="x1g")
                    hm_dma.dma_start(xg[ch][:, :npx],
                                     x1_d.ap()[128 * ch:128 * (ch + 1), r0:r1, :])
                off = 0
                while off < npx:
                    n = min(512, npx - off)
                    pshm = sps.tile([1, 512], F32, name="pshm", tag="ps", padded_shape=[128, 512])
                    for ch in range(2):
                        nc.tensor.matmul(pshm[:, :n], hmw[:, ch:ch + 1],
                                         xg[ch][:, off:off + n],
                                         start=(ch == 0), stop=(ch == 1))
                    nc.scalar.activation(hm_flat[0:1, r0 * WH + off:
                                                  r0 * WH + off + n],
                                         pshm[:, :n], AF.Sigmoid)
                    off += n

            # ---------------- NMS ----------------
            hm_pad = sm.tile([RH + 2, WH + 2], F32, name="hm_pad")
            nc.vector.memset(hm_pad[:], NEG)
            nc.gpsimd.dma_start(hm_pad[:, 1:WH + 1], hm_flat[0:1, :])
            # invalidate out-of-image halo rows (bias 0 on valid rows)
            nc.vector.tensor_scalar(hm_pad[:], hm_pad[:], hbias[:, :], None,
                                    op0=OP.add)
            m1 = sm.tile([RH + 2, WH + 1], F32, name="m1")
            nc.vector.tensor_tensor(m1[:], hm_pad[:, 0:WH + 1],
                                    hm_pad[:, 1:WH + 2], op=OP.max)
            hmx = sm.tile([RH + 2, WH], F32, name="hmx")
            nc.vector.tensor_tensor(hmx[:], m1[:, 0:WH], m1[:, 1:WH + 1],
                                    op=OP.max)
            vB = sm.tile([RH, WH], F32, name="vB")
            vC = sm.tile([RH, WH], F32, name="vC")
            hmc = sm.tile([RH, WH], F32, name="hmc")
            nc.gpsimd.dma_start(vB[:], hmx[1:RH + 1, :])
            nc.gpsimd.dma_start(vC[:], hmx[2:RH + 2, :])
            nc.gpsimd.dma_start(hmc[:], hm_pad[1:RH + 1, 1:WH + 1])
            vmax = sm.tile([RH, WH], F32, name="vmax")
            nc.vector.tensor_tensor(vmax[:], hmx[0:RH, :], vB[:], op=OP.max)
            nc.vector.tensor_tensor(vmax[:], vmax[:], vC[:], op=OP.max)
            kp = sm.tile([RH, WH], F32, name="kp")
            nc.vector.tensor_tensor(kp[:], vmax[:], hmc[:], op=OP.is_equal)
            heat = sm.tile([RH, WH], F32, name="heat")
            nc.vector.tensor_tensor(heat[:], kp[:], hmc[:], op=OP.mult)

            # ---------------- local top-8 ----------------
            flat = mid.tile([1, HPIX], F32, name="flat", tag="mid")
            nc.gpsimd.dma_start(flat[0:1, :], heat[:])
            v8 = sm.tile([1, 8], F32, name="v8")
            i8 = sm.tile([1, 8], U32, name="i8")
            nc.vector.max_with_indices(v8[:], i8[:], flat[:])
            if8 = sm.tile([1, 8], F32, name="if8")
            nc.vector.tensor_copy(if8[:], i8[:])
            gidx8 = sm.tile([1, 8], F32, name="gidx8")
            nc.vector.tensor_scalar(gidx8[:], if8[:], ibase[:, :], None,
                                    op0=OP.add)
            i8p = sm.tile([8, 1], U32, name="i8p")
            nc.gpsimd.dma_start(i8p[:, 0:1], i8[0:1, :])
            feat8 = sm.tile([8, C], F32, name="feat8")
            nc.gpsimd.indirect_dma_start(
                out=feat8[:], out_offset=None, in_=x1p_d.ap(),
                in_offset=bass.IndirectOffsetOnAxis(ap=i8p[:, :1], axis=0))

            # ---------------- allgather candidates ----------------
            agi_in = dram.tile([1, 8], F32, name="agi_in")
            agi_out = dram.tile([NCORE, 8], F32, name="agi_out",
                                addr_space="Shared")
            agf_in = dram.tile([8, C], F32, name="agf_in")
            agf_out = dram.tile([NCORE * 8, C], F32, name="agf_out",
                                addr_space="Shared")
            nc.gpsimd.dma_start(agi_in[:], gidx8[:])
            nc.gpsimd.dma_start(agf_in[:], feat8[:])
            nc.gpsimd.collective_compute(
                "AllGather", OP.bypass, replica_groups=[list(range(NCORE))],
                ins=[agi_in.opt()], outs=[agi_out.opt()])
            nc.gpsimd.collective_compute(
                "AllGather", OP.bypass, replica_groups=[list(range(NCORE))],
                ins=[agf_in.opt()], outs=[agf_out.opt()])

            # ---------------- conv 3x3 + relu ----------------
            W2 = WM + 2
            shifts = [(c_, y_, x_) for c_ in range(2) for y_ in range(3)
                      for x_ in range(3)]
            pairs = list(range(RM // 2))
            for g0 in range(0, len(pairs), 3):
                grp = pairs[g0:g0 + 3]
                pcs = []
                for p in grp:
                    pcv = cps.tile([128, WM], F32, name="pcv", tag="cv",
                                   padded_shape=[128, 512])
                    pcs.append(pcv)
                for si, (ch, dy, dx) in enumerate(shifts):
                    lhsT = wt_sb[:, (ch * 9 + dy * 3 + dx) * 64:
                                 (ch * 9 + dy * 3 + dx + 1) * 64]
                    for pcv, p in zip(pcs, grp):
                        ra = x0c[ch][:, (2 * p + dy) * W2 + dx:
                                     (2 * p + dy) * W2 + dx + WM]
                        rb = x0c[ch][:, (2 * p + 1 + dy) * W2 + dx:
                                     (2 * p + 1 + dy) * W2 + dx + WM]
                        nc.tensor.matmul(pcv[0:64, :], lhsT, ra,
                                         start=(si == 0), stop=(si == 17),
                                         skip_group_check=True)
                        nc.tensor.matmul(pcv[64:128, :], lhsT, rb,
                                         start=(si == 0), stop=(si == 17),
                                         skip_group_check=True)
                for pcv, p in zip(pcs, grp):
                    nc.scalar.activation(x_cat[0:64, (2 * p) * WM:
                                               (2 * p + 1) * WM],
                                         pcv[0:64, :], AF.Relu,
                                         bias=mbb[0:64, :])
                    oddst = oddp.tile([128, WM], BF16, name="oddst", tag="odd")
                    nc.vector.tensor_scalar(oddst[64:128, :], pcv[64:128, :],
                                            mbb[64:128, :], 0.0,
                                            op0=OP.add, op1=OP.max)
                    nc.sync.dma_start(
                        x_cat[0:64, (2 * p + 1) * WM:(2 * p + 2) * WM],
                        oddst[64:128, :])

            # ---------------- fp32 rescore of all 64 candidates ----------------
            fa = sm.tile([64, C], F32, name="fa")
            nc.gpsimd.dma_start(fa[:], agf_out[:])
            featsT = sm.tile([128, 128], F32, name="featsT")
            for ch in range(2):
                pst = tps.tile([128, 64], F32, name="pst", tag="tp", padded_shape=[128, 512])
                nc.tensor.transpose(pst[:], fa[:, 128 * ch:128 * (ch + 1)],
                                    ident[0:64, 0:64])
                nc.scalar.copy(featsT[:, 64 * ch:64 * (ch + 1)], pst[:])
            psc = sps.tile([1, 64], F32, name="psc", tag="ps", padded_shape=[128, 512])
            hmw32 = sm.tile([128, 2], F32, name="hmw32")
            nc.gpsimd.dma_start(hmw32[:], hmw_d.ap())
            for ch in range(2):
                nc.tensor.matmul(psc[:], hmw32[:, ch:ch + 1],
                                 featsT[:, 64 * ch:64 * (ch + 1)],
                                 start=(ch == 0), stop=(ch == 1))
            scoref = sm.tile([1, 64], F32, name="scoref")
            nc.scalar.activation(scoref[:], psc[:], AF.Sigmoid)

            # ---------------- global top-4 ----------------
            gv8 = sm.tile([1, 8], F32, name="gv8")
            gp8 = sm.tile([1, 8], U32, name="gp8")
            nc.vector.max_with_indices(gv8[:], gp8[:], scoref[:])
            nc.sync.dma_start(sco_d.ap(), gv8[0:1, 0:K])
            gp4 = sm.tile([K, 1], U32, name="gp4")
            nc.gpsimd.dma_start(gp4[:, 0:1], gp8[0:1, 0:K])
            gi = sm.tile([K, 1], F32, name="gi")
            nc.gpsimd.indirect_dma_start(
                out=gi[:], out_offset=None,
                in_=agi_out.opt().rearrange("a b -> (a b)").unsqueeze(1),
                in_offset=bass.IndirectOffsetOnAxis(ap=gp4[:, :1], axis=0))
            giu = sm.tile([K, 1], U32, name="giu")
            nc.vector.tensor_copy(giu[:], gi[:])
            nc.sync.dma_start(ind_d.ap(), giu[:, 0:1])
            fw = sm.tile([K, C], F32, name="fw")
            nc.gpsimd.indirect_dma_start(
                out=fw[:], out_offset=None, in_=agf_out.opt(),
                in_offset=bass.IndirectOffsetOnAxis(ap=gp4[:, :1], axis=0))

            # ---------------- params head at the 4 winners ----------------
            fwT = sm.tile([128, 2 * K], F32, name="fwT")
            for ch in range(2):
                psf = tps.tile([128, K], F32, name="psf", tag="tp", padded_shape=[128, 512])
                nc.tensor.transpose(psf[:], fw[:, 128 * ch:128 * (ch + 1)],
                                    ident[0:K, 0:K])
                nc.scalar.copy(fwT[:, K * ch:K * (ch + 1)], psf[:])
            psa = sps.tile([128, K], F32, name="psa", tag="ps", padded_shape=[128, 512])
            psb = sps.tile([6, K], F32, name="psb", tag="ps", padded_shape=[128, 512])
            for ch, pwc in enumerate((pw0, pw1)):
                nc.tensor.matmul(psa[:], pwc[:, 0:128],
                                 fwT[:, K * ch:K * (ch + 1)],
                                 start=(ch == 0), stop=(ch == 1))
            for ch, pwc in enumerate((pw0, pw1)):
                nc.tensor.matmul(psb[:], pwc[:, 128:NGEN],
                                 fwT[:, K * ch:K * (ch + 1)],
                                 start=(ch == 0), stop=(ch == 1))
            sel_a = sm.tile([128, K], F32, name="sel_a")
            sel_b = sm.tile([6, K], F32, name="sel_b")
            nc.vector.tensor_scalar(sel_a[:], psa[:], pb0[:, :], None, op0=OP.add)
            nc.vector.tensor_scalar(sel_b[:], psb[:], pb1[:, :], None, op0=OP.add)

            # head weights [67, 8]: cols 0:4 masks (rows 0:67 of sel),
            # cols 4:8 regs (rows 67:134 of sel)
            hd = sm.tile([NMASK, 2 * K], BF16, name="hd")
            nc.vector.tensor_copy(hd[0:NMASK, 0:K], sel_a[0:NMASK, :])
            nc.gpsimd.dma_start(hd[0:61, K:2 * K], sel_a[67:128, :])
            nc.gpsimd.dma_start(hd[61:67, K:2 * K], sel_b[0:6, :])

            # ---------------- dynamic heads ----------------
            for half in range(RM // 2):
                mrb = mrp.tile([8, 2 * WM], F32, name="mrb", tag="mr")
                for j in range(2):
                    r = 2 * half + j
                    psh = sps.tile([8, WM], F32, name="psh", tag="ps",
                                   padded_shape=[128, 512])
                    nc.tensor.matmul(psh[:], hd[:, :],
                                     x_cat[:, WM * r:WM * (r + 1)],
                                     start=True, stop=True)
                    if j == 0:
                        nc.scalar.copy(mrb[:, :WM], psh[:])
                    else:
                        nc.vector.tensor_copy(mrb[:, WM:], psh[:])
                nc.sync.dma_start(
                    mro_d.ap().rearrange("k r w -> k (r w)")
                    [:, 2 * WM * half:2 * WM * (half + 1)],
                    mrb[:])

            # ---------------- row-range MLP ----------------
            mt = big.tile([RM, 4 * WM], mlp_mm_dt, name="mt", tag="big")
            for k in range(K):
                mlp_dma.dma_start(mt[:, WM * k:WM * (k + 1)], mro_d.ap()[k])
            th = big.tile([128, 2048], F32, name="th", tag="big")
            h1 = th[0:64, 0:4 * WM]
            for k in range(K):
                psm = sps.tile([64, WM], F32, name="psm", tag="ps", padded_shape=[128, 512])
                nc.tensor.matmul(psm[:], w1loc[:, :], mt[:, WM * k:WM * (k + 1)],
                                 start=True, stop=True)
                nc.scalar.copy(h1[:, WM * k:WM * (k + 1)], psm[:])
            arh_in = dram.tile([64, 4 * WM], F32, name="arh_in")
            arh_out = dram.tile([64, 4 * WM], F32, name="arh_out",
                                addr_space="Shared")
            nc.gpsimd.dma_start(arh_in[:], h1)
            nc.gpsimd.collective_compute(
                "AllReduce", OP.add, replica_groups=[list(range(NCORE))],
                ins=[arh_in.opt()], outs=[arh_out.opt()])
            h1ar = th[64:128, 0:4 * WM]
            nc.gpsimd.dma_start(h1ar, arh_out[:])
            h1r = big.tile([64, 4 * WM], mlp_mm_dt, name="h1r", tag="big")
            nc.scalar.activation(h1r[:], h1ar, AF.Relu, bias=b1t[:, :])
            fr = mid.tile([2, 4 * WM], F32, name="fr", tag="mid")
            for k in range(K):
                psr = sps.tile([2, WM], F32, name="psr", tag="ps", padded_shape=[128, 512])
                nc.tensor.matmul(psr[:], w2t[:, :], h1r[:, WM * k:WM * (k + 1)],
                                 start=True, stop=True)
                nc.vector.tensor_scalar(fr[:, WM * k:WM * (k + 1)], psr[:],
                                        b2t[:, :], None, op0=OP.add)
            nc.sync.dma_start(fro_d.ap(), fr[:])

    nc.compile()
    return nc


_NC = None


def _get_nc():
    global _NC
    if _NC is None:
        _NC = _build()
    return _NC


def _prep_inputs(out0, out1, hm_w, hm_b, params_w, params_b, mb_w, mb_b,
                 mlp_w1, mlp_b1, mlp_w2, mlp_b2):
    bf = ml_dtypes.bfloat16
    o0p = np.zeros((C, HM + 2, WM + 2), dtype=bf)
    o0p[:, 1:HM + 1, 1:WM + 1] = out0[0].astype(bf)
    o1p = np.zeros((C, HH + 2, WH), dtype=np.float32)
    o1p[:, 1:HH + 1, :] = out1[0]
    x1p_full = np.ascontiguousarray(
        out1[0].transpose(1, 2, 0).reshape(HH * WH, C))

    wt = mb_w.transpose(1, 2, 3, 0).reshape(C, 9, 64)
    wt = wt.reshape(2, 128, 9, 64).transpose(1, 0, 2, 3).reshape(128, 1152)
    wt = np.ascontiguousarray(wt).astype(bf)
    mbb = np.tile(mb_b.reshape(64, 1), (2, 1)).astype(np.float32)
    hmw = np.ascontiguousarray(hm_w[0].reshape(2, 128).T).astype(np.float32)
    pw = np.ascontiguousarray(params_w.T.reshape(2, 128, NGEN)).astype(np.float32)
    pb = params_b.reshape(NGEN, 1).astype(np.float32)
    b1 = mlp_b1.reshape(64, 1).astype(np.float32)
    w2 = mlp_w2.astype(np.float32)
    b2 = mlp_b2.reshape(2, 1).astype(np.float32)

    xx = np.linspace(-1.0, 1.0, WM, dtype=np.float32)
    yy = np.linspace(-1.0, 1.0, HM, dtype=np.float32)
    ones = np.ones(PIX, dtype=np.float32)
    xxr = np.tile(xx, RM)

    in_maps = []
    for i in range(NCORE):
        hb = np.zeros((RH + 2, 1), np.float32)
        for r in range(RH + 2):
            g = RH * i - 1 + r
            if g < 0 or g >= HH:
                hb[r, 0] = NEG
        coords = np.stack([
            xxr,
            np.repeat(yy[RM * i:RM * (i + 1)], WM),
            ones]).astype(bf)
        in_maps.append({
            "x0": np.ascontiguousarray(o0p[:, RM * i:RM * i + RM + 2, :]),
            "x1": np.ascontiguousarray(o1p[:, RH * i:RH * i + RH + 2, :]),
            "x1p": np.ascontiguousarray(
                x1p_full[RH * WH * i:RH * WH * (i + 1)]),
            "hbias": hb,
            "ibase": np.array([[HPIX * i]], np.float32),
            "wt": wt, "mbb": mbb, "hmw": hmw, "pw": pw, "pb": pb,
            "coords": coords,
            "w1loc": np.ascontiguousarray(mlp_w1[RM * i:RM * (i + 1)]).astype(np.float32),
            "b1": b1, "w2": w2, "b2": b2,
        })
    return in_maps


def kernel(out0, out1, hm_w, hm_b, params_w, params_b, mb_w, mb_b,
           mlp_w1, mlp_b1, mlp_w2, mlp_b2, num_ins):
    assert int(num_ins) == K
    assert float(hm_b[0]) == 0.0
    nc = _get_nc()
    in_maps = _prep_inputs(out0, out1, hm_w, hm_b, params_w, params_b,
                           mb_w, mb_b, mlp_w1, mlp_b1, mlp_w2, mlp_b2)
    import os
    if os.environ.get("KERNEL_SIM"):
        from concourse.bass_interp import MultiCoreSim
        sim = MultiCoreSim(nc, num_cores=NCORE, trace=False,
                           require_finite=False, require_nnan=False)
        for c in range(NCORE):
            for name, arr in in_maps[c].items():
                sim.cores[c].tensor(name)[:] = arr
        sim.simulate(check_with_hw=False)
        res = [{nm: np.array(sim.cores[c].tensor(nm))
                for nm in ("mro", "sco", "ind", "fro")} for c in range(NCORE)]
    else:
        from concourse.bass_utils import run_bass_kernel_spmd
        res = run_bass_kernel_spmd(nc, in_maps, list(range(NCORE))).results

    masks0 = np.concatenate([res[i]["mro"][0:4] for i in range(NCORE)], axis=1)
    regs0 = np.concatenate([res[i]["mro"][4:8] for i in range(NCORE)], axis=1)
    scores = res[0]["sco"][0].astype(np.float32)
    inds = res[0]["ind"][0].astype(np.int32)
    feat_range = np.ascontiguousarray(
        res[0]["fro"].reshape(2, K, WM).transpose(1, 2, 0)).astype(np.float32)
    return scores, inds, regs0, masks0, feat_range
